# revision 14
# baseline (speedup 1.0000x reference)
"""Trainium2 Bass kernel for nn_AttentionBlock (B=2, N=2048, dim=1024, 16 heads x 64).

Sharding: 8 cores = 2 batches x 4 head-groups (4 heads per core, tensor-parallel
over heads for qkv/attention; the to_out projection is computed as per-core
partial sums over the local 256 hb-dims, gathered and added on host).

v4 design notes (evolves v3; same numerics, all bf16 matmuls):
  * All host-side DRAM layouts are per-partition contiguous (2-8KB runs) so
    DMA streams at line rate: x as [w][p][4096], weights as [p][2048],
    outputs as 16 q-tile chunks [p][1024] reassembled on host.
  * DMA order is need-driven across both queues (sync: x windows, w0 split
    in halves; gpsimd: wk, wq, wv, xw2 helper, wo).  The first k/q
    projection chains are split per e-half.  A PE warmup burst keeps the
    clock ramped through the DMA window.
  * Steps run W-MAJOR (window outer, head-pair inner): both head-pairs of a
    window finish 2 steps apart, so every window's output projection is a
    fused 2-it PSUM chain dripped mid-kernel (no w3 split, single output
    tensor) and np work spreads from slot ~39.
  * Within a slot the PE order is pv(g-1) -> drips -> st(g+2): pv and drips
    are already eligible when exp(g) starts, so the queue never head-of-line
    blocks on st's wait for exp(g) to free the PSUM bank.
  * Softmax normalization chain is on-chip-first: rowsum [1,1024] -> DVE
    reciprocal -> one DRAM hop for the partition-broadcast -> DVE multiply.
  * V projection runs full-width ([128c,256f], 16 groups of 8 matmuls).
  * Drip jobs carry (ready, deadline, pe_cost): slots pull the most urgent
    ready job while the slot budget allows (carrying deficit), and jobs at
    their deadline are force-emitted (emission order IS dependency order).
Matmuls are bf16 with fp32 accumulation; S^T pairs are PE row-tiled (auto
tile_position from base partitions 0/64) so both heads' QK^T run concurrently.
Softmax skips max-subtraction (logits ~N(0,1), exp safe in fp32).
"""

import ml_dtypes
import numpy as np

import concourse.bass as bass
import concourse.mybir as mybir
import concourse.tile as tile
from concourse.bass_utils import run_bass_kernel_spmd

B = 2
N = 2048
D = 1024
H = 16
DH = 64
HPC = 4  # heads per core
NCORES = 8
HB = HPC * DH  # 256: head-block width per core
NKT = N // 128  # 16 k-tiles
NW = 4  # 512-wide q-windows
NSTEP = 2 * NW  # (w, it) steps, w-major
NSLOT = NSTEP * NKT  # 128 global (step, kt) slots

f32 = mybir.dt.float32
bf16 = mybir.dt.bfloat16
EXP = mybir.ActivationFunctionType.Exp

_WAIT_CAP = 1


def _split_excess_waits(nc):
    """The walrus build in this container rejects instructions carrying more
    than a couple of sync-wait commands ("Too many sync wait commands" in
    CoreV3GenImpl setupSyncWait). Tile's semaphore assignment freely attaches
    several waits to one instruction. Hoist the excess onto dedicated
    single-wait NOPs inserted just before the instruction on the same engine
    (program order on that engine preserves the wait-before-execute
    semantics)."""
    f = nc.m.functions[0]
    for blk in f.blocks:
        out = []
        changed = False
        for inst in blk.instructions:
            si = inst.sync_info
            waits = list(si.on_wait) if si is not None and si.on_wait else []
            if len(waits) > _WAIT_CAP:
                changed = True
                for j, w in enumerate(waits[: -_WAIT_CAP]):
                    nop = mybir.InstNoOp(
                        name=f"{inst.name}-ws{j}",
                        engine=inst.engine,
                        sync_info=mybir.SyncInfo(on_wait=[w], on_update=[]),
                        bass_nofuse=True,
                    )
                    nc.register_instruction(nop)
                    out.append(nop)
                si.on_wait = waits[-_WAIT_CAP:]
            out.append(inst)
        if changed:
            blk.instructions = out


def _build_nc():
    nc = bass.Bass()
    # x per window: [w][p][e*n] with each partition's 8KB contiguous
    xTc_d = nc.dram_tensor("xTc", [NW, 128, 8 * 512], bf16, kind="ExternalInput")
    # weights packed [p][e*hb] / [p][i*d]: 2-4KB contiguous per partition
    wqT_d = nc.dram_tensor("wqT", [128, 8 * HB], bf16, kind="ExternalInput")
    wkT_d = nc.dram_tensor("wkT", [128, 8 * HB], bf16, kind="ExternalInput")
    wvT_d = nc.dram_tensor("wvT", [128, 8 * HB], bf16, kind="ExternalInput")
    woT_d = nc.dram_tensor("woT", [128, 2 * D], bf16, kind="ExternalInput")
    # output: 16 q-tile chunks [p][1024], host reassembles
    yf_d = nc.dram_tensor("yf", [16, 128, D], bf16, kind="ExternalOutput")

    with tile.TileContext(nc) as tc:
        with (
            tc.tile_pool(name="main", bufs=1) as main,
            tc.tile_pool(name="ptp", bufs=4) as ptp,
            tc.tile_pool(name="ysp", bufs=4) as ysp,
            tc.tile_pool(name="spp", bufs=2) as spp,
            tc.tile_pool(name="drm", bufs=2, space="DRAM") as drm,
            tc.tile_pool(name="aux", bufs=1, space="PSUM") as aux,
            tc.tile_pool(name="stp", bufs=2, space="PSUM") as stp,
            tc.tile_pool(name="otp", bufs=1, space="PSUM") as otp,
        ):
            # ---- persistent tensors ----
            qT = main.tile([128, 2, N], bf16)  # rows: head-pair dims for it
            kT = main.tile([128, 2, N], bf16)
            vaug = main.tile([128, NKT, HPC, DH + 1], bf16)  # [k%128, kt, h, d|1]
            ocat = main.tile([128, 2, N], bf16)  # O^T rows per it; cols q
            wo = main.tile([128, 2, D], bf16)
            xtw = [
                main.tile([128, 8, 512], bf16, name=f"xtw{w}") for w in range(NW)
            ]
            wq = main.tile([128, 8, HB], bf16)
            wk = main.tile([128, 8, HB], bf16)
            wv = main.tile([128, 8, HB], bf16)
            ones_t = main.tile([128, 1], bf16)
            import itertools as _it
            _auxcyc = _it.cycle(["qkv", "np"])

            nc.vector.memset(ones_t[:], 1.0)

            # ---- input DMA, arrival-ordered, big contiguous runs ----
            wkf = wk.rearrange("p e h -> p (e h)")
            wqf = wq.rearrange("p e h -> p (e h)")
            xf = [t.rearrange("p e n -> p (e n)") for t in xtw]
            # head-critical tensors split across BOTH queues so the first
            # projection chains' deps (wk/wq/xw0) land in parallel.
            nc.sync.dma_start(wkf[:, 0:1024], wkT_d[:, 0:1024])
            nc.gpsimd.dma_start(wqf[:, 0:1024], wqT_d[:, 0:1024])
            nc.sync.dma_start(xf[0][:, 0:2048], xTc_d[0, :, 0:2048])
            nc.gpsimd.dma_start(wkf[:, 1024:2048], wkT_d[:, 1024:2048])
            nc.sync.dma_start(wqf[:, 1024:2048], wqT_d[:, 1024:2048])
            nc.gpsimd.dma_start(xf[0][:, 2048:4096], xTc_d[0, :, 2048:4096])
            nc.gpsimd.dma_start(wv.rearrange("p e h -> p (e h)"), wvT_d[:])
            nc.sync.dma_start(xf[1], xTc_d[1])
            nc.gpsimd.dma_start(xf[2], xTc_d[2])
            nc.sync.dma_start(xf[3], xTc_d[3])
            nc.gpsimd.dma_start(wo.rearrange("p i o -> p (i o)"), woT_d[:])

            nc.vector.tensor_copy(
                vaug[:, :, :, DH : DH + 1],
                ones_t[:, :, None, None].to_broadcast([128, NKT, HPC, 1]),
            )

            # ---- PE warmup: dep-free matmuls through the DMA window ----
            garb = main.tile([128, 512], bf16)  # warmup fuel
            nc.vector.memset(garb[:], 0.0)
            warm_ps = aux.tile([128, 512], f32, tag=next(_auxcyc), name="warm")
            for i in range(14):
                nc.tensor.matmul(
                    warm_ps[0:128, :], lhsT=garb[:, 0:128], rhs=garb, start=True,
                    stop=True,
                )

            # ---- projection-group emitters ----
            _qkn = [0]

            def emit_qk_half(w_t, it, q4, half, ps=None):
                if ps is None:
                    _qkn[0] += 1
                    ps = aux.tile(
                        [128, 512], f32, tag=next(_auxcyc), name=f"qkps{_qkn[0]}"
                    )
                for eo in range(4 * half, 4 * half + 4):
                    nc.tensor.matmul(
                        ps,
                        lhsT=w_t[:, eo, it * 128 : (it + 1) * 128],
                        rhs=xtw[q4][:, eo, :],
                        start=(eo == 0),
                        stop=(eo == 7),
                    )
                return ps

            def emit_qk_group(dst, w_t, it, q4):
                ps = emit_qk_half(w_t, it, q4, 0)
                emit_qk_half(w_t, it, q4, 1, ps=ps)
                nc.vector.tensor_copy(dst[:, it, q4 * 512 : (q4 + 1) * 512], ps)

            def emit_v_group(nt):
                # full-width V projection: all 4 heads at once
                ps_full = aux.tile(
                    [128, 512], f32, tag=next(_auxcyc), name=f"vps{nt}"
                )
                ps = ps_full[:, 0:HB]
                for eo in range(8):
                    nc.tensor.matmul(
                        ps,
                        lhsT=xtw[nt // 4][:, eo, (nt % 4) * 128 : (nt % 4 + 1) * 128],
                        rhs=wv[:, eo, :],
                        start=(eo == 0),
                        stop=(eo == 7),
                    )
                nc.vector.tensor_copy(
                    vaug[:, nt, :, 0:DH],
                    ps.rearrange("p (h d) -> p h d", h=HPC),
                )

            # ---- steps, w-major: step s = (w, it) = (s // 2, s % 2) ----
            def step_of(g):
                s = g // NKT
                return s % 2, s // 2  # (it, w)

            st_tiles = {}  # g -> PSUM S^T tile
            pt_tiles = {}  # g -> SBUF P^T tile
            ot_tiles = {}  # s -> (ot_lo, ot_hi)
            stg_tiles = {}  # s -> staging [65, 2, 512]
            sst_tiles = {}  # s -> row-sum SBUF tile
            bcs = {}  # s -> broadcast reciprocal tile

            def emit_st(g):
                it, w = step_of(g)
                kt = g % NKT
                st2 = stp.tile([128, 2, 512], f32, tag="st", name=f"st{g}")
                for s in range(2):
                    nc.tensor.matmul(
                        st2[:, s, :],
                        lhsT=kT[s * 64 : s * 64 + 64, it, kt * 128 : (kt + 1) * 128],
                        rhs=qT[s * 64 : s * 64 + 64, it, w * 512 : (w + 1) * 512],
                        start=True,
                        stop=True,
                    )
                st_tiles[g] = st2

            def emit_exp(g):
                pt2 = ptp.tile([128, 2, 512], bf16, tag="pt", name=f"pt{g}")
                nc.scalar.activation(
                    pt2.rearrange("p s q -> p (s q)"),
                    st_tiles.pop(g).rearrange("p s q -> p (s q)"),
                    EXP,
                    scale=0.125,
                )
                pt_tiles[g] = pt2

            def emit_pv(g):
                it, w = step_of(g)
                s_idx = g // NKT
                kt = g % NKT
                if kt == 0:
                    ot_tiles[s_idx] = (
                        otp.tile([128, 512], f32, tag="otlo", name=f"otlo{s_idx}"),
                        otp.tile([128, 512], f32, tag="othi", name=f"othi{s_idx}"),
                    )
                pt2 = pt_tiles.pop(g)
                for s, ot in zip(range(2), ot_tiles[s_idx]):
                    nc.tensor.matmul(
                        ot[0:65, :],
                        lhsT=vaug[:, kt, 2 * it + s, :],
                        rhs=pt2[:, s, :],
                        start=(kt == 0),
                        stop=(kt == NKT - 1),
                    )

            def emit_evac(s_idx):
                # fast PSUM release: one [65,512] staging copy per head
                ot_lo, ot_hi = ot_tiles.pop(s_idx)
                stg = spp.tile([65, 2, 512], f32, tag="stg", name=f"stg{s_idx}")
                nc.vector.tensor_copy(stg[:, 0, :], ot_lo[0:65, :])
                nc.vector.tensor_copy(stg[:, 1, :], ot_hi[0:65, :])
                stg_tiles[s_idx] = stg

            def emit_split(s_idx):
                it, w = s_idx % 2, s_idx // 2
                q0 = w * 512
                stg = stg_tiles.pop(s_idx)
                sst = spp.tile([1, 1024], f32, tag="sst", name=f"sst{s_idx}")
                for s in range(2):
                    nc.vector.tensor_copy(
                        ocat[s * 64 : s * 64 + 64, it, q0 : q0 + 512], stg[0:64, s, :]
                    )
                    nc.vector.tensor_copy(
                        sst[0:1, s * 512 : (s + 1) * 512], stg[64:65, s, :]
                    )
                sst_tiles[s_idx] = sst

            def emit_chain(s_idx):
                # reciprocal of both heads' row sums: bounce through DRAM to
                # spread the 1024 values over all 128 DVE lanes (a [1,1024]
                # single-partition reciprocal measures ~6.4ns/elem), then
                # bounce back and broadcast-replicate each head's 512 values
                # across its 64 ocat rows via stride-0 DMA.
                sst = sst_tiles.pop(s_idx)
                stmp = drm.tile([1, 1024], f32, tag="stmp")
                nc.sync.dma_start(stmp, sst)
                spk = spp.tile([128, 8], f32, tag="spk")
                nc.sync.dma_start(spk, stmp.rearrange("a (p j) -> (a p) j", p=128))
                rpk = spp.tile([128, 8], f32, tag="rpk")
                nc.vector.reciprocal(rpk, spk)
                rtmp = drm.tile([1, 1024], f32, tag="rtmp")
                nc.sync.dma_start(rtmp.rearrange("a (p j) -> (a p) j", p=128), rpk)
                bc32 = spp.tile([128, 512], f32, tag="bc32")
                rv = rtmp.rearrange("a (s q) -> (a s) q", s=2)
                for s in range(2):
                    nc.sync.dma_start(
                        bc32[s * 64 : (s + 1) * 64, :],
                        rv[s : s + 1, :].to_broadcast([64, 512]),
                    )
                bcs[s_idx] = bc32

            def emit_norm(s_idx):
                it, w = s_idx % 2, s_idx // 2
                q0 = w * 512
                osl = ocat[:, it, q0 : q0 + 512]
                nc.vector.tensor_mul(osl, osl, bcs.pop(s_idx))

            def emit_np_fused(w, qt, late=False):
                # fused output projection for global q-tile 4w+qt: both oc
                # halves accumulate 2-it chains, one [128,1024] ys, one DMA.
                r0 = w * 512 + qt * 128
                ys = ysp.tile([128, 2, 512], bf16, tag="ys")
                for oc in range(2):
                    yp = aux.tile(
                        [128, 512], f32, tag=next(_auxcyc), name=f"yp{w}_{qt}_{oc}"
                    )
                    for it in range(2):
                        nc.tensor.matmul(
                            yp,
                            lhsT=ocat[:, it, r0 : r0 + 128],
                            rhs=wo[:, it, oc * 512 : (oc + 1) * 512],
                            start=(it == 0),
                            stop=(it == 1),
                        )
                    if late and oc == 1:
                        nc.scalar.copy(ys[:, oc, :], yp)
                    else:
                        nc.vector.tensor_copy(ys[:, oc, :], yp)
                nc.sync.dma_start(
                    yf_d[4 * w + qt], ys.rearrange("p a b -> p (a b)")
                )


            # ---- deadline-driven drip scheduling ----
            # Emission order IS dependency order under Tile, so each job's
            # deadline is the last slot before its first consumer is emitted;
            # jobs at deadline are force-emitted regardless of budget.
            EXP_NS = 1110.0
            MAND_NS = 700.0
            CREDIT_CAP = 2400.0
            jobs = []

            def job(ready, deadline, cost, fn, *a, **kw):
                jobs.append(
                    [ready, deadline, cost, (lambda: fn(*a, **kw)), len(jobs)]
                )

            pin = {g: [] for g in range(NSLOT + 1)}

            def at(g, fn, *a, **kw):
                pin[g].append(lambda: fn(*a, **kw))

            # consumers: st(G) is emitted at slot G-2 (before slot G-2's
            # pulls), so a kT/qT group feeding first slot G must be emitted
            # by slot G-3.  pv(G) is emitted at slot G+1, so v(nt) (first
            # consumed by pv at slot nt) must be emitted by slot nt.
            # kT it0 kw1-3: st(4j) -> deadline 4j-3
            job(0, 1, 2400, emit_qk_group, kT, wk, 0, 1)
            job(1, 5, 2400, emit_qk_group, kT, wk, 0, 2)
            job(4, 9, 2400, emit_qk_group, kT, wk, 0, 3)
            # kT it1 kw0-3: st(16+4j) -> deadline 13+4j
            job(2, 13, 2400, emit_qk_group, kT, wk, 1, 0)
            job(6, 17, 2400, emit_qk_group, kT, wk, 1, 1)
            job(8, 21, 2400, emit_qk_group, kT, wk, 1, 2)
            job(10, 25, 2400, emit_qk_group, kT, wk, 1, 3)
            # qT it1 w0: st(16) -> deadline 13
            job(3, 12, 2400, emit_qk_group, qT, wq, 1, 0)
            # qT w1-3 both its: st(32w...) -> deadline 32w-3 / 32w+13
            job(5, 29, 2400, emit_qk_group, qT, wq, 0, 1)
            job(12, 45, 2400, emit_qk_group, qT, wq, 1, 1)
            job(16, 61, 2400, emit_qk_group, qT, wq, 0, 2)
            job(24, 77, 2400, emit_qk_group, qT, wq, 1, 2)
            job(32, 93, 2400, emit_qk_group, qT, wq, 0, 3)
            job(40, 109, 2400, emit_qk_group, qT, wq, 1, 3)
            # V groups: v(nt) deadline nt (xw arrival staggers ready)
            for nt in range(NKT):
                ready = max(0, [0, 2, 4, 7][nt // 4])
                job(ready, nt, 1500, emit_v_group, nt)
            # np fused: window w ready after norm(2w+1) pin at (2w+2)*16+6
            for w in range(3):
                for qt in range(4):
                    job((2 * w + 2) * 16 + 9 + qt, 114 + 4 * w + qt, 1300,
                        emit_np_fused, w, qt)

            # boundary bookkeeping: split + chain + norm pinned after each
            # step boundary (evac runs in the main loop at the boundary)
            for s_idx in range(NSTEP - 1):
                gb = (s_idx + 1) * NKT
                at(gb + 1, emit_split, s_idx)
                at(gb + 2, emit_chain, s_idx)
                at(gb + 8, emit_norm, s_idx)

            # ---- upfront groups (split e-halves, interleaved) ----
            k_ps = emit_qk_half(wk, 0, 0, 0)
            q_ps = emit_qk_half(wq, 0, 0, 0)
            emit_qk_half(wk, 0, 0, 1, ps=k_ps)
            nc.vector.tensor_copy(kT[:, 0, 0:512], k_ps)
            emit_qk_half(wq, 0, 0, 1, ps=q_ps)
            nc.vector.tensor_copy(qT[:, 0, 0:512], q_ps)
            emit_st(0)
            emit_st(1)

            jobs.sort(key=lambda j: (j[1], j[0], j[4]))
            state = {"budget": 0.0}

            def pull_jobs(g, force_only=False):
                # force jobs at deadline; then most-urgent ready while budget
                while True:
                    best = None
                    for j in jobs:
                        if j[0] <= g and (j[1] <= g or
                                          (not force_only and state["budget"] > 0)):
                            best = j
                            break
                    if best is None:
                        return
                    jobs.remove(best)
                    best[3]()
                    state["budget"] = max(state["budget"] - best[2], -4000.0)

            for g in range(NSLOT):
                emit_exp(g)
                state["budget"] = min(
                    state["budget"] + EXP_NS - MAND_NS, CREDIT_CAP
                )
                if g >= 1:
                    emit_pv(g - 1)
                if g % NKT == 0 and g > 0:
                    emit_evac(g // NKT - 1)
                for fn in pin[g]:
                    fn()
                pull_jobs(g)
                if g + 2 < NSLOT:
                    emit_st(g + 2)
            # tail
            emit_pv(NSLOT - 1)
            emit_evac(NSTEP - 1)
            emit_split(NSTEP - 1)
            emit_chain(NSTEP - 1)
            emit_norm(NSTEP - 1)
            # drain any leftovers (np w2 stragglers), then w3 projections
            for j in list(jobs):
                jobs.remove(j)
                j[3]()
            for qt in range(4):
                emit_np_fused(3, qt, late=(qt % 2 == 1))

    _split_excess_waits(nc)
    return nc


_CACHED_NC = None


def _get_nc():
    global _CACHED_NC
    if _CACHED_NC is None:
        _CACHED_NC = _build_nc()
    return _CACHED_NC


def _make_in_maps(x, w_qkv, w_out):
    b16 = ml_dtypes.bfloat16

    def c(a):
        return np.ascontiguousarray(a.astype(b16))

    # x -> [w][p][e*n]: window-chunked, per-partition contiguous 8KB runs
    xTc = []
    for b in range(B):
        xT = x[b].T  # [D, N]
        xc = (
            xT.reshape(8, 128, NW, 512)
            .transpose(2, 1, 0, 3)
            .reshape(NW, 128, 8 * 512)
        )
        xTc.append(c(xc))

    in_maps = []
    for core in range(NCORES):
        b = core // (NCORES // B)
        hb = core % (NCORES // B)
        rows = slice(hb * HB, (hb + 1) * HB)

        # pack [p][e*h]: partition p holds w^T rows {e*128+p} contiguously
        def pack_w(wT):  # wT: [D, HB]
            return c(wT.reshape(8, 128, HB).transpose(1, 0, 2).reshape(128, 8 * HB))

        woT = w_out[:, rows].T  # [HB, D]
        wo_pack = c(woT.reshape(2, 128, D).transpose(1, 0, 2).reshape(128, 2 * D))
        in_maps.append(
            {
                "xTc": xTc[b],
                "wqT": pack_w(w_qkv[0 * D : 1 * D][rows].T),
                "wkT": pack_w(w_qkv[1 * D : 2 * D][rows].T),
                "wvT": pack_w(w_qkv[2 * D : 3 * D][rows].T),
                "woT": wo_pack,
            }
        )
    return in_maps


def kernel(x, w_qkv, w_out, b_out, _trace=False, _trace_kwargs=None):
    x = np.asarray(x, dtype=np.float32)
    w_qkv = np.asarray(w_qkv, dtype=np.float32)
    w_out = np.asarray(w_out, dtype=np.float32)
    b_out = np.asarray(b_out, dtype=np.float32)

    in_maps = _make_in_maps(x, w_qkv, w_out)

    nc = _get_nc()
    kwargs = {}
    if _trace:
        kwargs["trace"] = True
        if _trace_kwargs:
            kwargs.update(_trace_kwargs)
    res = run_bass_kernel_spmd(nc, in_maps, core_ids=list(range(NCORES)), **kwargs)

    out = np.zeros((B, N, D), dtype=np.float32)
    for core in range(NCORES):
        b = core // (NCORES // B)
        r = res.results[core]
        out[b] += r["yf"].astype(np.float32).reshape(N, D)
    out += b_out[None, None, :]
    kernel._last_result = res
    return out
